# revision 3
# baseline (speedup 1.0000x reference)
"""ConvLSTM-Trans block (UpSample2x -> ConvLSTM2D -> BN -> concat skip) on 8 TRN2 cores.

Sharding: 8 cores = 4 batches x 2 H-halves. The time recurrence is sequential,
so each core runs the full T=16 loop on its half of the image with a shrinking
halo (the halo region is recomputed redundantly instead of exchanged, so cores
never communicate). Bottom halves are vertically flipped (data + conv kernels)
so every core runs the *same* program with its open boundary at the bottom.

On-device layout is channel-major: conv = 9 shifted matmuls accumulating in
PSUM, with the stacked [x_up(64ch); h(64ch)] tile as the K=128 contraction dim
(fp32r, full PE rate at N>=256). Gates/cell math run on ACT+DVE. Upsampling,
output transpose back to NHWC, and the skip concat are host-side numpy.
"""

import numpy as np

import concourse.mybir as mybir
import concourse.tile as tile
from concourse import bacc
from concourse.bass_utils import run_bass_kernel_spmd

f32 = mybir.dt.float32
f32r = mybir.dt.float32r
ALU = mybir.AluOpType
ACT = mybir.ActivationFunctionType

N_CORES = 8
T = 16
F = 64           # hidden channels
CIN = 64
CMAP = 32
HU = 96          # upsampled H/W
WU = 96
ROWS = 64        # local state rows per core (48 own + 16 halo)
COLS = 98        # 96 + 1 zero-pad col each side
SROWS = 65       # 1 zero-pad row on the closed edge + 64 data rows
OWN = 48         # rows this core owns in the output
BN_EPS = 1e-3

_CACHED_NC = None


def _row_chunks(e):
    """Split e rows into chunks of <=5 rows (<=480 cols), each >=3 rows so the
    matmul free dim stays >=256 (fp32r full-rate threshold)."""
    nfull, rem = divmod(e, 5)
    sizes = [5] * nfull
    if rem:
        sizes.append(rem)
        if rem in (1, 2):
            sizes[-2] = 3
            sizes[-1] = rem + 2
    out = []
    r0 = 0
    for s in sizes:
        out.append((r0, s))
        r0 += s
    return out


def _build_program():
    nc = bacc.Bacc("TRN2", target_bir_lowering=False, debug=False,
                   num_devices=N_CORES)

    xd = nc.dram_tensor("x", [T, CIN, ROWS, COLS], f32r, kind="ExternalInput").ap()
    h0d = nc.dram_tensor("h0", [F, ROWS, COLS], f32r, kind="ExternalInput").ap()
    c0d = nc.dram_tensor("c0", [F, ROWS * 96], f32, kind="ExternalInput").ap()
    wd = nc.dram_tensor("wst", [9, 128, 256], f32r, kind="ExternalInput").ap()
    cd = nc.dram_tensor("consts", [128, 5], f32, kind="ExternalInput").ap()
    od = nc.dram_tensor("outc", [T, F, OWN * 96], f32, kind="ExternalOutput").ap()

    with tile.TileContext(nc) as tc:
        with (
            tc.tile_pool(name="pp", bufs=1) as pp,
            tc.tile_pool(name="gp", bufs=3) as gp,
            tc.tile_pool(name="op", bufs=2) as op,
            tc.tile_pool(name="zp", bufs=6, space="PSUM") as zp,
        ):
            # persistent state
            S = [pp.tile([128, SROWS * COLS], f32r, tag=f"S{i}", name=f"S{i}")
                 for i in range(2)]
            Sv = [s.rearrange("p (r c) -> p r c", c=COLS) for s in S]
            lgc = pp.tile([128, ROWS * 96], f32, tag="lgc")
            wsb = pp.tile([128, 9, 256], f32r, tag="wsb")
            csb = pp.tile([128, 5], f32, tag="csb")

            nc.gpsimd.memset(S[0][:].bitcast(f32), 0.0)
            nc.gpsimd.memset(S[1][:].bitcast(f32), 0.0)
            nc.sync.dma_start(out=wsb[:], in_=wd.rearrange("t k m -> k t m"))
            nc.sync.dma_start(out=csb[:], in_=cd[:])
            nc.sync.dma_start(out=Sv[0][64:128, 1:65, :], in_=h0d[:])
            nc.sync.dma_start(out=lgc[64:128, :], in_=c0d[:])

            b01 = csb[:, 0:1]            # 0.2*b[i,f] + 0.5
            bg = csb[0:64, 1:2]          # b[g]
            bo = csb[64:128, 1:2]        # 0.2*b[o] + 0.5
            bns = csb[64:128, 2:3]       # gamma/sqrt(var+eps)
            bnb = csb[64:128, 3:4]       # beta - mean*scale
            alg = csb[0:64, 4:5]         # 0.3 (leaky slope, Prelu alpha)
            alc = csb[64:128, 4:5]

            for t in range(1, T + 1):
                cur, nxt = S[(t - 1) % 2], S[t % 2]
                vcur, vnxt = Sv[(t - 1) % 2], Sv[t % 2]
                r_in = ROWS - (t - 1)    # valid conv-input rows in cur
                e = ROWS - t             # rows of h_t to compute
                # x_t -> cur[0:64]; contiguous (pad cols come zeroed from host)
                nc.sync.dma_start(out=vcur[0:64, 1:1 + r_in, :],
                                  in_=xd[t - 1, :, 0:r_in, :])

                for (row0, nr) in _row_chunks(e):
                    N = nr * 96
                    off = row0 * 96
                    z0 = zp.tile([128, 480], f32, tag="z")
                    z1 = zp.tile([128, 480], f32, tag="z")
                    for mc, z in ((0, z0), (1, z1)):
                        zv = z[:, :N].rearrange("p (r c) -> p r c", c=96)
                        for tap in range(9):
                            ky, kx = divmod(tap, 3)
                            nc.tensor.matmul(
                                zv[:, :, :],
                                wsb[:, tap, mc * 128:(mc + 1) * 128],
                                vcur[:, row0 + ky: row0 + ky + nr, kx: kx + 96],
                                start=(tap == 0), stop=(tap == 8),
                            )
                    # gates: z0 = [i; f], z1 = [g; o]; lgc = [lg; c]
                    a0 = gp.tile([128, 480], f32, tag="a0")
                    ho = gp.tile([128, 480], f32, tag="ho")
                    mmu = gp.tile([128, 480], f32, tag="mmu")
                    mmv = gp.tile([128, 480], f32, tag="mmv")
                    lc = gp.tile([128, 480], f32, tag="lc")
                    # hard_sigmoid pre-clip for i,f: relu(0.2 z + 0.2 b + 0.5)
                    nc.scalar.activation(a0[:, :N], z0[:, :N], ACT.Relu,
                                         bias=b01, scale=0.2)
                    # leaky g (exact, Prelu w/ alpha AP), + bias
                    nc.scalar.activation(lgc[0:64, off:off + N], z1[0:64, :N],
                                         ACT.Prelu, bias=bg, scale=1.0, alpha=alg)
                    # hard_sigmoid pre-clip for o
                    nc.scalar.activation(ho[64:128, :N], z1[64:128, :N], ACT.Relu,
                                         bias=bo, scale=0.2)
                    # i_term = min(a0_i,1)*lg -> parts 64:128 ; f_term = min(a0_f,1)*c
                    nc.vector.scalar_tensor_tensor(
                        mmu[64:128, :N], a0[0:64, :N], 1.0, lgc[0:64, off:off + N],
                        ALU.min, ALU.mult)
                    nc.vector.scalar_tensor_tensor(
                        mmv[64:128, :N], a0[64:128, :N], 1.0, lgc[64:128, off:off + N],
                        ALU.min, ALU.mult)
                    # c_new
                    nc.vector.tensor_add(out=lgc[64:128, off:off + N],
                                         in0=mmu[64:128, :N], in1=mmv[64:128, :N])
                    # leaky(c_new)
                    nc.scalar.activation(lc[64:128, :N], lgc[64:128, off:off + N],
                                         ACT.Prelu, alpha=alc)
                    # h = min(ho,1)*lc -> next stacked tile (f32r rounding on write)
                    nc.vector.scalar_tensor_tensor(
                        vnxt[64:128, row0 + 1: row0 + 1 + nr, 1:97],
                        ho[64:128, :N].rearrange("p (r c) -> p r c", c=96),
                        1.0,
                        lc[64:128, :N].rearrange("p (r c) -> p r c", c=96),
                        ALU.min, ALU.mult)

                # BN on own rows -> staging -> DRAM (channel-major; host transposes)
                ob = op.tile([128, OWN * 96], f32, tag="ob")
                obv = ob.rearrange("p (r c) -> p r c", c=96)
                for j in range(4):
                    nc.scalar.activation(
                        obv[64:128, 12 * j: 12 * j + 12, :],
                        vnxt[64:128, 12 * j + 1: 12 * j + 13, 1:97],
                        ACT.Identity, bias=bnb, scale=bns)
                nc.sync.dma_start(out=od[t - 1], in_=ob[64:128, :])

    nc.compile()
    return nc


def _get_nc():
    global _CACHED_NC
    if _CACHED_NC is None:
        _CACHED_NC = _build_program()
    return _CACHED_NC


def _prep_inputs(inputs, h0, c0, Wx, Wh, b, gamma, beta, mmean, mvar):
    """Build the 8 per-core input maps (numpy only)."""
    s = (gamma.astype(np.float64) / np.sqrt(mvar.astype(np.float64) + BN_EPS))
    bn_bias = (beta.astype(np.float64) - mmean.astype(np.float64) * s)

    in_maps = []
    for core in range(N_CORES):
        bi, half = divmod(core, 2)
        flip = half == 1

        xb = inputs[bi]                                        # [T,48,48,64]
        xu = np.repeat(np.repeat(xb, 2, axis=1), 2, axis=2)    # [T,96,96,64]
        if flip:
            xw = xu[:, 32:96][:, ::-1]
        else:
            xw = xu[:, 0:64]
        x_np = np.zeros((T, CIN, ROWS, COLS), np.float32)
        x_np[:, :, :, 1:97] = xw.transpose(0, 3, 1, 2)

        h0w = h0[bi, 32:96][::-1] if flip else h0[bi, 0:64]
        h0_np = np.zeros((F, ROWS, COLS), np.float32)
        h0_np[:, :, 1:97] = h0w.transpose(2, 0, 1)

        c0w = c0[bi, 32:96][::-1] if flip else c0[bi, 0:64]
        c0_np = np.ascontiguousarray(c0w.transpose(2, 0, 1)).reshape(F, ROWS * 96)

        Wxc = Wx[::-1] if flip else Wx
        Whc = Wh[::-1] if flip else Wh
        wst = np.empty((9, 128, 256), np.float32)
        for ky in range(3):
            for kx in range(3):
                wst[ky * 3 + kx, 0:64] = Wxc[ky, kx]
                wst[ky * 3 + kx, 64:128] = Whc[ky, kx]

        consts = np.zeros((128, 5), np.float32)
        consts[:, 0] = 0.2 * b[0:128] + 0.5
        consts[0:64, 1] = b[128:192]
        consts[64:128, 1] = 0.2 * b[192:256] + 0.5
        consts[64:128, 2] = s
        consts[64:128, 3] = bn_bias
        consts[:, 4] = 0.3

        in_maps.append({
            "x": x_np, "h0": h0_np, "c0": np.ascontiguousarray(c0_np),
            "wst": wst, "consts": consts,
        })
    return in_maps


def _assemble(results, map_):
    out = np.empty((4, T, HU, WU, CMAP + F), np.float32)
    out[:, :, :, :, 0:CMAP] = map_[:, None]
    for core in range(N_CORES):
        bi, half = divmod(core, 2)
        oc = results[core]["outc"].reshape(T, F, OWN, 96).transpose(0, 2, 3, 1)
        if half == 1:
            oc = oc[:, ::-1]
        out[bi, :, half * OWN:(half + 1) * OWN, :, CMAP:] = oc
    return out


def run(inputs, h0, c0, map_, Wx, Wh, b, gamma, beta, mmean, mvar, trace=False):
    nc = _get_nc()
    in_maps = _prep_inputs(inputs, h0, c0, Wx, Wh, b, gamma, beta, mmean, mvar)
    res = run_bass_kernel_spmd(nc, in_maps, list(range(N_CORES)), trace=trace)
    out = _assemble(res.results, np.asarray(map_, dtype=np.float32))
    return out, res


def kernel(inputs, h0, c0, map_, Wx, Wh, b, gamma, beta, mmean, mvar):
    out, _ = run(np.asarray(inputs, np.float32), np.asarray(h0, np.float32),
                 np.asarray(c0, np.float32), np.asarray(map_, np.float32),
                 np.asarray(Wx, np.float32), np.asarray(Wh, np.float32),
                 np.asarray(b, np.float32), np.asarray(gamma, np.float32),
                 np.asarray(beta, np.float32), np.asarray(mmean, np.float32),
                 np.asarray(mvar, np.float32))
    return out
